# revision 1
# baseline (speedup 1.0000x reference)
import numpy as np
import jax
import jax.numpy as jnp
from jax.sharding import Mesh, PartitionSpec as P
try:
    from jax.experimental.shard_map import shard_map
except ImportError:
    from jax.shard_map import shard_map

# Problem: CapsNet dynamic routing (ClassifierCaps)
#   x: [256, 1152, 8] fp32, W: [10, 1152, 8, 16] fp32
#   out: v [10, 256, 1, 1, 16] fp32
# Sharding: batch (B=256) split 8 ways -> 32 per core; W replicated.

B, N, CIN, COUT, K = 256, 1152, 8, 16, 10
NCORES = 8
ROUTING_ITERATIONS = 3

_compiled = None


def _squash(s):
    sq = jnp.sum(s * s, axis=-1, keepdims=True)
    return (sq / (1.0 + sq)) * s / jnp.sqrt(sq)


def _routing_shard(x, W):
    # x: [B/8, N, CIN] local shard; W: [K, N, CIN, COUT] replicated
    u_hat = jnp.einsum('bnc,kncd->kbnd', x, W)  # [K, b, N, D]
    b = jnp.zeros_like(u_hat)
    v = None
    for it in range(ROUTING_ITERATIONS):
        c = jax.nn.softmax(b, axis=2)
        s = jnp.sum(c * u_hat, axis=2, keepdims=True)  # [K, b, 1, D]
        v = _squash(s)
        if it < ROUTING_ITERATIONS - 1:
            a = jnp.sum(u_hat * v, axis=-1, keepdims=True)
            b = b + a
    return v[:, :, :, None, :]  # [K, b, 1, 1, D]


def _get_compiled():
    global _compiled
    if _compiled is None:
        devs = jax.devices()[:NCORES]
        mesh = Mesh(np.array(devs), ('dp',))
        f = shard_map(
            _routing_shard,
            mesh=mesh,
            in_specs=(P('dp', None, None), P(None, None, None, None)),
            out_specs=P(None, 'dp', None, None, None),
        )
        _compiled = jax.jit(f)
    return _compiled


def kernel(x: np.ndarray, W: np.ndarray) -> np.ndarray:
    f = _get_compiled()
    out = f(jnp.asarray(x, dtype=jnp.float32), jnp.asarray(W, dtype=jnp.float32))
    return np.asarray(jax.device_get(out), dtype=np.float32)

